# revision 13
# baseline (speedup 1.0000x reference)
"""GAT (2-layer, 8-head) Trainium2 kernel over 8 NeuronCores.

Strategy (edge-cut node sharding):
- Pad N 50000->50176 = 8 shards * 6272. Core c owns nodes [6272c, 6272(c+1)).
- Host: sort edges by dest, bucket into 128-node blocks. Within a block, edges
  are split lo (col < 32768) / hi (col >= 32768) because the batched DMA_GATHER
  instruction takes int16 indices; the hi gather uses a table base offset of
  32768 rows. Each section is padded to whole 128-edge chunks (pad slots gather
  row 0 and carry row_local=200 so their one-hot column is zero).
- Per-edge f_dst values never touch DRAM: f_dst stays SBUF-resident per block
  and is expanded edge-wise with tiny PE matmuls against a host-shipped
  TRANSPOSED one-hot (OHT[n,e] = [row_local(e)==n]), removing half of all
  gather descriptors (the SWDGE descriptor rate ~4-7ns/desc is the kernel's
  main bottleneck).
- Device per core:
  Phase A (f16 PE): Wh|f_dst|f_src = x_shard @ [W_cat|WA_dst|WA_src]. Store
    table row [Wh f16 512 | f_src_hi 8 | f_src_lo 8 | pad->640]; f_dst -> SBUF.
  AllGather table -> full [50176, 640] f16.
  Phase B (per 128-dst-node block): dma_gather (lo+hi pieces, round-robin over
    4 SWDGE queues) fetches table[col] for the whole block; pfd = OHT.T@f_dst
    per chunk; p = exp(leaky(pfd+f_src)) (f16); per-chunk PE matmuls accumulate
    num += onehot.T @ (p*Wh[col]), den += onehot.T @ p in PSUM.
    h = elu(num/den); transpose h (f16) via PE; Wh2|f2 = h @ [W_out|...];
    write layer-2 table shard [SHARD, 128]; fd2 edge-expansion (OHT.T@fd2)
    lands in a resident SBUF strip for phase C.
  AllGather layer-2 table [50176, 128] f16.
  Phase C: same batched-gather scatter loop with 64-wide messages + fused
    denominator column; logit dst-part read from the resident strip;
    out = num2/den2.
- Softmax needs no segment-max: logits are O(6) so exp never overflows, and
  normalization commutes with the scatter-sum (divide once per node).
"""
import os
import sys
sys.path.insert(0, "/opt/trn_rl_repo")
import numpy as np

import concourse.tile as tile
from concourse import bass, bacc, mybir
from concourse.bass_utils import run_bass_kernel_spmd
from concourse.masks import make_identity

N, E = 50000, 800000
NFEAT, NHID, NHEADS, NCLASS = 512, 64, 8, 64
ALPHA = 0.2
NC = 8
NPAD = 50176
SHARD = NPAD // NC        # 6272
BLK = 128
NBPC = SHARD // BLK       # 49 blocks per core
KT = NFEAT // 128         # 4 k-tiles
DW1 = NFEAT + 16          # 528: Wh | src_hi | src_lo
TW1 = 640                 # table-1 row (1280B, %256B for dma_gather)
DW2 = NCLASS + 2          # 66:  Wh2 | src_hi | src_lo
TW2 = 128                 # table-2 row (256B)
HSPL = 3200               # shard rows in section A (A/B split keeps idx < 32768)

f16d, f32d, i16d = mybir.dt.float16, mybir.dt.float32, mybir.dt.int16

LAST_EXEC_NS = None
LAST_RESULTS = None
_BUILD_CACHE = {}


def _wrap16(lst):
    """DMA_GATHER index layout: element i at [i%16, i//16], replicated x8."""
    return np.tile(lst.reshape(-1, 16).T, (8, 1))


def _preprocess(row, col):
    order = np.argsort(row, kind="stable")
    row_s = row[order].astype(np.int64)
    col_s = col[order].astype(np.int64)
    counts = np.bincount(row_s // BLK, minlength=NPAD // BLK)
    starts = np.concatenate([[0], np.cumsum(counts)])
    nb = NPAD // BLK
    src_c = col_s // SHARD
    src_r = col_s % SHARD
    in_a = src_r < HSPL
    idx_a = src_c * HSPL + src_r
    idx_b = src_c * (SHARD - HSPL) + (src_r - HSPL)
    nlo = np.zeros(nb, np.int64)
    for b in range(nb):
        nlo[b] = int(in_a[starts[b]:starts[b + 1]].sum())
    nhi = counts - nlo
    cpl = int(((nlo + 127) // 128).max())
    cph = int(((nhi + 127) // 128).max())
    cpe = cpl + cph
    cilo = np.zeros((NC, 128, NBPC * cpl * 8), np.int16)
    cihi = np.zeros((NC, 128, NBPC * cph * 8), np.int16)
    rl = np.full((NC, 128, NBPC * cpe), 200.0, np.float16)
    oht = np.zeros((NC, NBPC, 128, cpe * 128), np.float16)
    ohh = np.zeros((NC, NBPC, 128, cpe * 128), np.float16)
    iota = np.arange(128)
    for b in range(nb):
        c, bl = divmod(b, NBPC)
        s, e = starts[b], starts[b + 1]
        rloc = row_s[s:e] - b * BLK
        m = in_a[s:e]
        lo_c, lo_r = idx_a[s:e][m], rloc[m]
        hi_c, hi_r = idx_b[s:e][~m], rloc[~m]
        lst = np.zeros(cpl * 128, np.int16)
        lst[:len(lo_c)] = lo_c
        cilo[c, :, bl * cpl * 8:(bl + 1) * cpl * 8] = _wrap16(lst)
        rlo = np.full(cpl * 128, 200.0, np.float16)
        rlo[:len(lo_r)] = lo_r
        lst2 = np.zeros(cph * 128, np.int16)
        lst2[:len(hi_c)] = hi_c
        cihi[c, :, bl * cph * 8:(bl + 1) * cph * 8] = _wrap16(lst2)
        rhi = np.full(cph * 128, 200.0, np.float16)
        rhi[:len(hi_r)] = hi_r
        rl[c, :, bl * cpe:bl * cpe + cpl] = rlo.reshape(cpl, 128).T
        rl[c, :, bl * cpe + cpl:(bl + 1) * cpe] = rhi.reshape(cph, 128).T
        rf = np.concatenate([rlo, rhi])
        oht[c, bl] = (iota[:, None] == rf[None, :]).astype(np.float16)
        rfs = rf.reshape(cpe, 128)
        for cc in range(cpe):
            ohh[c, bl][:, cc * 128:(cc + 1) * 128] = (
                rfs[cc][:, None] == iota[None, :]).astype(np.float16)
    return cilo, cihi, rl, oht, ohh, cpl, cph


MAXCH = 8  # chunks per dma_gather piece (1024 idx = HW descriptor-ring cap)


def _build(cpl, cph):
    key = (cpl, cph)
    if key in _BUILD_CACHE:
        return _BUILD_CACHE[key]
    cpe = cpl + cph
    nc = bacc.Bacc("TRN2", target_bir_lowering=False, debug=False,
                   enable_asserts=True, num_devices=NC, num_swdge_queues=4)
    xt = nc.dram_tensor("xt", [NBPC * KT, 128, 128], f16d, kind="ExternalInput")
    w1 = nc.dram_tensor("w1", [KT * 128, DW1], f16d, kind="ExternalInput")
    w2 = nc.dram_tensor("w2", [KT * 128, DW2], f16d, kind="ExternalInput")
    cilo = nc.dram_tensor("cilo", [128, NBPC * cpl * 8], i16d, kind="ExternalInput")
    cihi = nc.dram_tensor("cihi", [128, NBPC * cph * 8], i16d, kind="ExternalInput")
    rl = nc.dram_tensor("rl", [128, NBPC * cpe], f16d, kind="ExternalInput")
    ohtT = nc.dram_tensor("ohtT", [NBPC, 128, cpe * 128], f16d,
                          kind="ExternalInput")
    ohhT = nc.dram_tensor("ohhT", [NBPC, 128, cpe * 128], f16d,
                          kind="ExternalInput")
    out = nc.dram_tensor("out", [SHARD, NCLASS], f32d, kind="ExternalOutput")

    AF, ALU = mybir.ActivationFunctionType, mybir.AluOpType

    qctr = [0]

    def gather_pieces(dst, table, idx_t, icol0, nch, elem):
        a = 0
        while a < nch:
            b = min(a + MAXCH, nch)
            nc.gpsimd.dma_gather(
                out_ap=dst[:, a * elem:b * elem]
                    .rearrange("p (c e) -> p c e", e=elem),
                in_ap=table,
                idxs_ap=idx_t[:, icol0 + a * 8:icol0 + b * 8],
                num_idxs=(b - a) * 128, num_idxs_reg=(b - a) * 128,
                elem_size=elem, queue_num=qctr[0] % 4)
            qctr[0] += 1
            a = b

    with tile.TileContext(nc) as tc:
        with tc.tile_pool(name="res", bufs=1) as res, \
             tc.tile_pool(name="dram", bufs=1, space="DRAM") as drp:
            tab1s = drp.tile([SHARD, TW1], f16d)
            tab1a = drp.tile([NC * HSPL, TW1], f16d, addr_space="Shared")
            tab1b = drp.tile([NC * (SHARD - HSPL), TW1], f16d,
                             addr_space="Shared")
            tab2s = drp.tile([SHARD, TW2], f16d)
            tab2a = drp.tile([NC * HSPL, TW2], f16d, addr_space="Shared")
            tab2b = drp.tile([NC * (SHARD - HSPL), TW2], f16d,
                             addr_space="Shared")

            w1_t = res.tile([128, KT * DW1], f16d)
            w2_t = res.tile([128, KT * DW2], f16d)
            for k in range(KT):
                nc.sync.dma_start(out=w1_t[:, k * DW1:(k + 1) * DW1],
                                  in_=w1[k * 128:(k + 1) * 128, :])
                nc.sync.dma_start(out=w2_t[:, k * DW2:(k + 1) * DW2],
                                  in_=w2[k * 128:(k + 1) * 128, :])
            cilo_t = res.tile([128, NBPC * cpl * 8], i16d)
            cihi_t = res.tile([128, NBPC * cph * 8], i16d)
            rl_t = res.tile([128, NBPC * cpe], f16d)
            nc.sync.dma_start(out=cilo_t[:], in_=cilo[:, :])
            nc.sync.dma_start(out=cihi_t[:], in_=cihi[:, :])
            nc.sync.dma_start(out=rl_t[:], in_=rl[:, :])
            ident = res.tile([128, 128], f16d)
            make_identity(nc, ident[:])
            fd_sb = res.tile([128, NBPC * 8], f16d)
            fd2e_sb = res.tile([128, NBPC * cpe], f32d)

            # ---------------- Phase A ----------------
            with nc.named_scope("phaseA"), \
                 tc.tile_pool(name="pa", bufs=3) as pa, \
                 tc.tile_pool(name="ppa", bufs=2, space="PSUM") as ppa:
                for nt in range(NBPC):
                    rows = slice(nt * 128, (nt + 1) * 128)
                    psA = ppa.tile([128, 512], f32d, tag="psA")
                    psB = ppa.tile([128, 16], f32d, tag="psB")
                    xk4 = pa.tile([128, KT * 128], f16d, tag="xk4")
                    for k in range(KT):
                        nc.sync.dma_start(out=xk4[:, k * 128:(k + 1) * 128],
                                          in_=xt[nt * KT + k, :, :])
                    for k in range(KT):
                        nc.tensor.matmul(out=psA[:],
                                         lhsT=xk4[:, k * 128:(k + 1) * 128],
                                         rhs=w1_t[:, k * DW1:k * DW1 + 512],
                                         start=(k == 0), stop=(k == KT - 1))
                        nc.tensor.matmul(out=psB[:],
                                         lhsT=xk4[:, k * 128:(k + 1) * 128],
                                         rhs=w1_t[:, k * DW1 + 512:(k + 1) * DW1],
                                         start=(k == 0), stop=(k == KT - 1))
                    whf = pa.tile([128, DW1], f16d, tag="whf")
                    nc.vector.tensor_copy(out=whf[:, :512], in_=psA[:])
                    nc.vector.tensor_copy(out=whf[:, 512:520], in_=psB[:, 8:16])
                    nc.vector.tensor_tensor(out=whf[:, 520:528], in0=psB[:, 8:16],
                                            in1=whf[:, 512:520], op=ALU.subtract)
                    nc.vector.tensor_copy(out=fd_sb[:, nt * 8:(nt + 1) * 8],
                                          in_=psB[:, 0:8])
                    nc.sync.dma_start(out=tab1s[rows, 0:DW1], in_=whf[:])

            with nc.named_scope("ag1"):
                nc.gpsimd.collective_compute(
                    "AllGather", ALU.bypass, replica_groups=[list(range(NC))],
                    ins=[tab1s[0:HSPL, :]],
                    outs=[tab1a[:].rearrange("(c r) d -> c r d", c=NC)])
                nc.gpsimd.collective_compute(
                    "AllGather", ALU.bypass, replica_groups=[list(range(NC))],
                    ins=[tab1s[HSPL:SHARD, :]],
                    outs=[tab1b[:].rearrange("(c r) d -> c r d", c=NC)])

            # ---------------- Phase B ----------------
            with nc.named_scope("phaseB"), \
                 tc.tile_pool(name="pb", bufs=2) as pb, \
                 tc.tile_pool(name="ppb", bufs=2, space="PSUM") as ppb, \
                 tc.tile_pool(name="ppf", bufs=1, space="PSUM") as ppf, \
                 tc.tile_pool(name="ppt", bufs=1, space="PSUM") as ppt:
                for bl in range(NBPC):
                    rows = slice(bl * 128, (bl + 1) * 128)
                    pnum = ppb.tile([128, 512], f32d, tag="pnum")
                    pden = ppb.tile([128, 8], f32d, tag="pden")
                    G = pb.tile([128, cpe * TW1], f16d, tag="G")
                    OH = pb.tile([128, cpe * 128], f16d, tag="OH")
                    OHT = pb.tile([128, cpe * 128], f16d, tag="OHT")
                    nc.sync.dma_start(out=OHT[:], in_=ohtT[bl, :, :])
                    nc.sync.dma_start(out=OH[:], in_=ohhT[bl, :, :])
                    gather_pieces(G[:, :cpl * TW1], tab1a[:, :],
                                  cilo_t, bl * cpl * 8, cpl, TW1)
                    gather_pieces(G[:, cpl * TW1:], tab1b[:, :],
                                  cihi_t, bl * cph * 8, cph, TW1)
                    pfd = ppf.tile([128, cpe * 8], f32d, tag="pfd")
                    for c in range(cpe):
                        nc.tensor.matmul(out=pfd[:, c * 8:(c + 1) * 8],
                                         lhsT=OHT[:, c * 128:(c + 1) * 128],
                                         rhs=fd_sb[:, bl * 8:(bl + 1) * 8],
                                         start=True, stop=True)
                    s1 = pb.tile([128, cpe * 8], f32d, tag="s1")
                    e1 = pb.tile([128, cpe * 8], f32d, tag="e1")
                    p16 = pb.tile([128, cpe * 8], f16d, tag="p16")
                    Gr = G[:].rearrange("p (c d) -> p c d", d=TW1)
                    nc.vector.tensor_tensor(
                        out=s1[:].rearrange("p (c f) -> p c f", c=cpe),
                        in0=Gr[:, :, 512:520],
                        in1=Gr[:, :, 520:528], op=ALU.add)
                    nc.vector.tensor_tensor(out=e1[:], in0=s1[:], in1=pfd[:],
                                            op=ALU.add)
                    nc.vector.tensor_scalar_mul(s1[:], e1[:], ALPHA)
                    nc.vector.tensor_tensor(out=e1[:], in0=e1[:],
                                            in1=s1[:], op=ALU.max)
                    nc.scalar.activation(out=p16[:], in_=e1[:], func=AF.Exp)
                    R = pb.tile([128, cpe * 512], f16d, tag="R")
                    spl = (cpe * 2) // 3
                    nc.vector.tensor_tensor(
                        out=R[:, :spl * 512]
                            .rearrange("p (c e f) -> p c e f", c=spl, e=8),
                        in0=Gr[:, :spl, 0:512]
                            .rearrange("p c (e f) -> p c e f", e=8),
                        in1=p16[:, :spl * 8]
                            .rearrange("p (c h) -> p c h", c=spl)
                            .to_broadcast([128, spl, 8, 64]),
                        op=ALU.mult)
                    nc.gpsimd.tensor_tensor(
                        out=R[:, spl * 512:]
                            .rearrange("p (c e f) -> p c e f", c=cpe - spl, e=8),
                        in0=Gr[:, spl:, 0:512]
                            .rearrange("p c (e f) -> p c e f", e=8),
                        in1=p16[:, spl * 8:]
                            .rearrange("p (c h) -> p c h", c=cpe - spl)
                            .to_broadcast([128, cpe - spl, 8, 64]),
                        op=ALU.mult)
                    for i in range(cpe):
                        nc.tensor.matmul(out=pnum[:],
                                         lhsT=OH[:, i * 128:(i + 1) * 128],
                                         rhs=R[:, i * 512:(i + 1) * 512],
                                         start=(i == 0), stop=(i == cpe - 1))
                        nc.tensor.matmul(out=pden[:],
                                         lhsT=OH[:, i * 128:(i + 1) * 128],
                                         rhs=p16[:, i * 8:(i + 1) * 8],
                                         start=(i == 0), stop=(i == cpe - 1))
                    # epilogue: h = elu(num/den), transpose, layer-2 tables
                    dcl = pb.tile([128, 8], f32d, tag="dcl")
                    nc.vector.tensor_scalar_max(dcl[:], pden[:], 1e-30)
                    nc.vector.reciprocal(out=dcl[:], in_=dcl[:])
                    ha = pb.tile([128, 512], f16d, tag="ha")
                    nc.vector.tensor_tensor(
                        out=ha[:].rearrange("p (e f) -> p e f", e=8),
                        in0=pnum[:].rearrange("p (e f) -> p e f", e=8),
                        in1=dcl[:].to_broadcast([128, 8, 64]),
                        op=ALU.mult)
                    hm = pb.tile([128, 512], f16d, tag="hm")
                    nc.vector.tensor_scalar_min(hm[:], ha[:], 0.0)
                    nc.scalar.activation(out=hm[:], in_=hm[:], func=AF.Exp)
                    nc.vector.tensor_scalar_sub(hm[:], hm[:], 1.0)
                    h16 = pb.tile([128, 512], f16d, tag="h16")
                    nc.vector.tensor_tensor(out=h16[:], in0=hm[:], in1=ha[:],
                                            op=ALU.max)
                    ps2 = ppt.tile([128, DW2], f32d, tag="ps2")
                    for k in range(KT):
                        pt = ppt.tile([128, 128], f16d, tag="pt")
                        nc.tensor.transpose(out=pt[:],
                                            in_=h16[:, k * 128:(k + 1) * 128],
                                            identity=ident[:])
                        ht = pb.tile([128, 128], f16d, tag="ht")
                        nc.vector.tensor_copy(out=ht[:], in_=pt[:])
                        nc.tensor.matmul(out=ps2[:], lhsT=ht[:],
                                         rhs=w2_t[:, k * DW2:(k + 1) * DW2],
                                         start=(k == 0), stop=(k == KT - 1))
                    t2 = pb.tile([128, DW2], f16d, tag="t2")
                    nc.vector.tensor_copy(out=t2[:, 0:64], in_=ps2[:, 0:64])
                    nc.vector.tensor_copy(out=t2[:, 64:65], in_=ps2[:, 65:66])
                    nc.vector.tensor_tensor(out=t2[:, 65:66], in0=ps2[:, 65:66],
                                            in1=t2[:, 64:65], op=ALU.subtract)
                    fd2 = pb.tile([128, 1], f16d, tag="fd2")
                    nc.vector.tensor_copy(out=fd2[:], in_=ps2[:, 64:65])
                    pfd2 = ppf.tile([128, cpe], f32d, tag="pfd2")
                    for c in range(cpe):
                        nc.tensor.matmul(out=pfd2[:, c:c + 1],
                                         lhsT=OHT[:, c * 128:(c + 1) * 128],
                                         rhs=fd2[:],
                                         start=True, stop=True)
                    nc.vector.tensor_copy(
                        out=fd2e_sb[:, bl * cpe:(bl + 1) * cpe], in_=pfd2[:])
                    nc.sync.dma_start(out=tab2s[rows, 0:DW2], in_=t2[:])

            with nc.named_scope("ag2"):
                nc.gpsimd.collective_compute(
                    "AllGather", ALU.bypass, replica_groups=[list(range(NC))],
                    ins=[tab2s[0:HSPL, :]],
                    outs=[tab2a[:].rearrange("(c r) d -> c r d", c=NC)])
                nc.gpsimd.collective_compute(
                    "AllGather", ALU.bypass, replica_groups=[list(range(NC))],
                    ins=[tab2s[HSPL:SHARD, :]],
                    outs=[tab2b[:].rearrange("(c r) d -> c r d", c=NC)])

            # ---------------- Phase C ----------------
            with nc.named_scope("phaseC"), \
                 tc.tile_pool(name="pc", bufs=3) as pc, \
                 tc.tile_pool(name="ppc", bufs=2, space="PSUM") as ppc:
                for bl in range(NBPC):
                    rows = slice(bl * 128, (bl + 1) * 128)
                    ps3 = ppc.tile([128, 65], f32d, tag="ps3")
                    G2 = pc.tile([128, cpe * TW2], f16d, tag="G2")
                    OH2 = pc.tile([128, cpe * 128], f16d, tag="OH2")
                    nc.sync.dma_start(out=OH2[:], in_=ohhT[bl, :, :])
                    gather_pieces(G2[:, :cpl * TW2], tab2a[:, :],
                                  cilo_t, bl * cpl * 8, cpl, TW2)
                    gather_pieces(G2[:, cpl * TW2:], tab2b[:, :],
                                  cihi_t, bl * cph * 8, cph, TW2)
                    e2 = pc.tile([128, cpe], f32d, tag="e2")
                    p2 = pc.tile([128, cpe], f16d, tag="p2")
                    G2r = G2[:].rearrange("p (c d) -> p c d", d=TW2)
                    nc.vector.tensor_tensor(
                        out=e2[:].rearrange("p (c o) -> p c o", o=1),
                        in0=G2r[:, :, 64:65],
                        in1=G2r[:, :, 65:66], op=ALU.add)
                    nc.vector.tensor_tensor(
                        out=e2[:], in0=e2[:],
                        in1=fd2e_sb[:, bl * cpe:(bl + 1) * cpe], op=ALU.add)
                    t2c = pc.tile([128, cpe], f32d, tag="t2c")
                    nc.vector.tensor_scalar_mul(t2c[:], e2[:], ALPHA)
                    nc.vector.tensor_tensor(out=e2[:], in0=e2[:],
                                            in1=t2c[:], op=ALU.max)
                    nc.scalar.activation(out=p2[:], in_=e2[:], func=AF.Exp)
                    R2 = pc.tile([128, cpe * 65], f16d, tag="R2")
                    R2r = R2[:].rearrange("p (c d) -> p c d", d=65)
                    nc.vector.tensor_tensor(
                        out=R2r[:, :, 0:64],
                        in0=G2r[:, :, 0:64],
                        in1=p2[:].to_broadcast([128, cpe, 64]),
                        op=ALU.mult)
                    nc.vector.tensor_copy(
                        out=R2r[:, :, 64:65],
                        in_=p2[:].rearrange("p (c o) -> p c o", o=1))
                    for i in range(cpe):
                        nc.tensor.matmul(out=ps3[:],
                                         lhsT=OH2[:, i * 128:(i + 1) * 128],
                                         rhs=R2[:, i * 65:(i + 1) * 65],
                                         start=(i == 0), stop=(i == cpe - 1))
                    d2c = pc.tile([128, 1], f32d, tag="d2c")
                    nc.vector.tensor_scalar_max(d2c[:], ps3[:, 64:65], 1e-30)
                    nc.vector.reciprocal(out=d2c[:], in_=d2c[:])
                    o = pc.tile([128, 64], f32d, tag="o")
                    nc.vector.tensor_tensor(
                        out=o[:].rearrange("p (c f) -> p c f", c=1),
                        in0=ps3[:, 0:64].rearrange("p (c f) -> p c f", c=1),
                        in1=d2c[:].to_broadcast([128, 1, 64]),
                        op=ALU.mult)
                    nc.sync.dma_start(out=out[rows, :], in_=o[:])

    nc.compile()
    _BUILD_CACHE[key] = nc
    return nc


def kernel(**inputs):
    global LAST_EXEC_NS, LAST_RESULTS
    x = inputs["x"].astype(np.float32)
    row = inputs["row"].astype(np.int64)
    col = inputs["col"].astype(np.int64)
    W, a = inputs["W"].astype(np.float32), inputs["a"].astype(np.float32)
    W_out = inputs["W_out"].astype(np.float32)
    a_out = inputs["a_out"].astype(np.float32)

    cilo, cihi, rl, oht, ohh, cpl, cph = _preprocess(row, col)
    cpe = cpl + cph

    W_cat = np.concatenate([W[h] for h in range(NHEADS)], axis=1)
    WA_dst = np.stack([W[h] @ a[h, :NHID] for h in range(NHEADS)], 1)
    WA_src = np.stack([W[h] @ a[h, NHID:] for h in range(NHEADS)], 1)
    w1_np = np.concatenate([W_cat, WA_dst, WA_src], 1).astype(np.float16)
    w2_np = np.concatenate([W_out, (W_out @ a_out[:NCLASS])[:, None],
                            (W_out @ a_out[NCLASS:])[:, None]], 1).astype(np.float16)

    x_pad = np.zeros((NPAD, NFEAT), np.float16)
    x_pad[:N] = x

    nc = _build(cpl, cph)

    in_maps = []
    for c in range(NC):
        xs = x_pad[c * SHARD:(c + 1) * SHARD]            # [6272, 512]
        xt = (xs.reshape(NBPC, 128, KT, 128)             # [nt, n, k, f]
                .transpose(0, 2, 3, 1)                   # [nt, k, f, n]
                .reshape(NBPC * KT, 128, 128)).copy()
        in_maps.append({"xt": xt, "w1": w1_np, "w2": w2_np,
                        "cilo": cilo[c], "cihi": cihi[c],
                        "rl": rl[c], "ohtT": oht[c], "ohhT": ohh[c]})

    trace = bool(int(os.environ.get("GAT_TRACE", "0")))
    res = run_bass_kernel_spmd(nc, in_maps, list(range(NC)), trace=trace,
                               trace_cores=list(range(NC)) if trace else None)
    LAST_EXEC_NS = res.exec_time_ns
    LAST_RESULTS = res
    outs = [res.results[c]["out"] for c in range(NC)]
    return np.concatenate(outs, 0)[:N].astype(np.float32)


# revision 14
# speedup vs baseline: 1.0593x; 1.0593x over previous
"""GAT (2-layer, 8-head) Trainium2 kernel over 8 NeuronCores.

Strategy (edge-cut node sharding):
- Pad N 50000->50176 = 8 shards * 6272. Core c owns nodes [6272c, 6272(c+1)).
- Host: sort edges by dest, bucket into 128-node blocks. Within a block, edges
  are split lo (col < 32768) / hi (col >= 32768) because the batched DMA_GATHER
  instruction takes int16 indices; the hi gather uses a table base offset of
  32768 rows. Each section is padded to whole 128-edge chunks (pad slots gather
  row 0 and carry row_local=200 so their one-hot column is zero).
- Per-edge f_dst values never touch DRAM: f_dst stays SBUF-resident per block
  and is expanded edge-wise with tiny PE matmuls against a host-shipped
  TRANSPOSED one-hot (OHT[n,e] = [row_local(e)==n]), removing half of all
  gather descriptors (the SWDGE descriptor rate ~4-7ns/desc is the kernel's
  main bottleneck).
- Device per core:
  Phase A (f16 PE): Wh|f_dst|f_src = x_shard @ [W_cat|WA_dst|WA_src]. Store
    table row [Wh f16 512 | f_src_hi 8 | f_src_lo 8 | pad->640]; f_dst -> SBUF.
  AllGather table -> full [50176, 640] f16.
  Phase B (per 128-dst-node block): dma_gather (lo+hi pieces, round-robin over
    4 SWDGE queues) fetches table[col] for the whole block; pfd = OHT.T@f_dst
    per chunk; p = exp(leaky(pfd+f_src)) (f16); per-chunk PE matmuls accumulate
    num += onehot.T @ (p*Wh[col]), den += onehot.T @ p in PSUM.
    h = elu(num/den); transpose h (f16) via PE; Wh2|f2 = h @ [W_out|...];
    write layer-2 table shard [SHARD, 128]; fd2 edge-expansion (OHT.T@fd2)
    lands in a resident SBUF strip for phase C.
  AllGather layer-2 table [50176, 128] f16.
  Phase C: same batched-gather scatter loop with 64-wide messages + fused
    denominator column; logit dst-part read from the resident strip;
    out = num2/den2.
- Softmax needs no segment-max: logits are O(6) so exp never overflows, and
  normalization commutes with the scatter-sum (divide once per node).
"""
import os
import sys
sys.path.insert(0, "/opt/trn_rl_repo")
import numpy as np

import concourse.tile as tile
from concourse import bass, bacc, mybir
from concourse.bass_utils import run_bass_kernel_spmd
from concourse.masks import make_identity

N, E = 50000, 800000
NFEAT, NHID, NHEADS, NCLASS = 512, 64, 8, 64
ALPHA = 0.2
NC = 8
NPAD = 50176
SHARD = NPAD // NC        # 6272
BLK = 128
NBPC = SHARD // BLK       # 49 blocks per core
KT = NFEAT // 128         # 4 k-tiles
DW1 = NFEAT + 16          # 528: Wh | src_hi | src_lo
TW1 = 640                 # table-1 row (1280B, %256B for dma_gather)
DW2 = NCLASS + 2          # 66:  Wh2 | src_hi | src_lo
TW2 = 128                 # table-2 row (256B)
HSPL = 2760               # shard rows in section A (sized so section A <= 8 chunks)

f16d, f32d, i16d = mybir.dt.float16, mybir.dt.float32, mybir.dt.int16

LAST_EXEC_NS = None
LAST_RESULTS = None
_BUILD_CACHE = {}


def _wrap16(lst):
    """DMA_GATHER index layout: element i at [i%16, i//16], replicated x8."""
    return np.tile(lst.reshape(-1, 16).T, (8, 1))


def _preprocess(row, col):
    order = np.argsort(row, kind="stable")
    row_s = row[order].astype(np.int64)
    col_s = col[order].astype(np.int64)
    counts = np.bincount(row_s // BLK, minlength=NPAD // BLK)
    starts = np.concatenate([[0], np.cumsum(counts)])
    nb = NPAD // BLK
    src_c = col_s // SHARD
    src_r = col_s % SHARD
    in_a = src_r < HSPL
    idx_a = src_c * HSPL + src_r
    idx_b = src_c * (SHARD - HSPL) + (src_r - HSPL)
    nlo = np.zeros(nb, np.int64)
    for b in range(nb):
        nlo[b] = int(in_a[starts[b]:starts[b + 1]].sum())
    nhi = counts - nlo
    cpl = int(((nlo + 127) // 128).max())
    cph = int(((nhi + 127) // 128).max())
    cpe = cpl + cph
    cilo = np.zeros((NC, 128, NBPC * cpl * 8), np.int16)
    cihi = np.zeros((NC, 128, NBPC * cph * 8), np.int16)
    rl = np.full((NC, 128, NBPC * cpe), 200.0, np.float16)
    oht = np.zeros((NC, NBPC, 128, cpe * 128), np.float16)
    ohh = np.zeros((NC, NBPC, 128, cpe * 128), np.float16)
    iota = np.arange(128)
    for b in range(nb):
        c, bl = divmod(b, NBPC)
        s, e = starts[b], starts[b + 1]
        rloc = row_s[s:e] - b * BLK
        m = in_a[s:e]
        lo_c, lo_r = idx_a[s:e][m], rloc[m]
        hi_c, hi_r = idx_b[s:e][~m], rloc[~m]
        lst = np.zeros(cpl * 128, np.int16)
        lst[:len(lo_c)] = lo_c
        cilo[c, :, bl * cpl * 8:(bl + 1) * cpl * 8] = _wrap16(lst)
        rlo = np.full(cpl * 128, 200.0, np.float16)
        rlo[:len(lo_r)] = lo_r
        lst2 = np.zeros(cph * 128, np.int16)
        lst2[:len(hi_c)] = hi_c
        cihi[c, :, bl * cph * 8:(bl + 1) * cph * 8] = _wrap16(lst2)
        rhi = np.full(cph * 128, 200.0, np.float16)
        rhi[:len(hi_r)] = hi_r
        rl[c, :, bl * cpe:bl * cpe + cpl] = rlo.reshape(cpl, 128).T
        rl[c, :, bl * cpe + cpl:(bl + 1) * cpe] = rhi.reshape(cph, 128).T
        rf = np.concatenate([rlo, rhi])
        oht[c, bl] = (iota[:, None] == rf[None, :]).astype(np.float16)
        rfs = rf.reshape(cpe, 128)
        for cc in range(cpe):
            ohh[c, bl][:, cc * 128:(cc + 1) * 128] = (
                rfs[cc][:, None] == iota[None, :]).astype(np.float16)
    return cilo, cihi, rl, oht, ohh, cpl, cph


MAXCH = 8  # chunks per dma_gather piece (1024 idx = HW descriptor-ring cap)


def _build(cpl, cph):
    key = (cpl, cph)
    if key in _BUILD_CACHE:
        return _BUILD_CACHE[key]
    cpe = cpl + cph
    nc = bacc.Bacc("TRN2", target_bir_lowering=False, debug=False,
                   enable_asserts=True, num_devices=NC, num_swdge_queues=4)
    xt = nc.dram_tensor("xt", [NBPC * KT, 128, 128], f16d, kind="ExternalInput")
    w1 = nc.dram_tensor("w1", [KT * 128, DW1], f16d, kind="ExternalInput")
    w2 = nc.dram_tensor("w2", [KT * 128, DW2], f16d, kind="ExternalInput")
    cilo = nc.dram_tensor("cilo", [128, NBPC * cpl * 8], i16d, kind="ExternalInput")
    cihi = nc.dram_tensor("cihi", [128, NBPC * cph * 8], i16d, kind="ExternalInput")
    rl = nc.dram_tensor("rl", [128, NBPC * cpe], f16d, kind="ExternalInput")
    ohtT = nc.dram_tensor("ohtT", [NBPC, 128, cpe * 128], f16d,
                          kind="ExternalInput")
    ohhT = nc.dram_tensor("ohhT", [NBPC, 128, cpe * 128], f16d,
                          kind="ExternalInput")
    out = nc.dram_tensor("out", [SHARD, NCLASS], f32d, kind="ExternalOutput")

    AF, ALU = mybir.ActivationFunctionType, mybir.AluOpType

    qctr = [0]

    def gather_pieces(dst, table, idx_t, icol0, nch, elem):
        a = 0
        while a < nch:
            b = min(a + MAXCH, nch)
            nc.gpsimd.dma_gather(
                out_ap=dst[:, a * elem:b * elem]
                    .rearrange("p (c e) -> p c e", e=elem),
                in_ap=table,
                idxs_ap=idx_t[:, icol0 + a * 8:icol0 + b * 8],
                num_idxs=(b - a) * 128, num_idxs_reg=(b - a) * 128,
                elem_size=elem, queue_num=qctr[0] % 4)
            qctr[0] += 1
            a = b

    with tile.TileContext(nc) as tc:
        with tc.tile_pool(name="res", bufs=1) as res, \
             tc.tile_pool(name="dram", bufs=1, space="DRAM") as drp:
            tab1s = drp.tile([SHARD, TW1], f16d)
            tab1a = drp.tile([NC * HSPL, TW1], f16d, addr_space="Shared")
            tab1b = drp.tile([NC * (SHARD - HSPL), TW1], f16d,
                             addr_space="Shared")
            tab2s = drp.tile([SHARD, TW2], f16d)
            tab2a = drp.tile([NC * HSPL, TW2], f16d, addr_space="Shared")
            tab2b = drp.tile([NC * (SHARD - HSPL), TW2], f16d,
                             addr_space="Shared")

            w1_t = res.tile([128, KT * DW1], f16d)
            w2_t = res.tile([128, KT * DW2], f16d)
            for k in range(KT):
                nc.sync.dma_start(out=w1_t[:, k * DW1:(k + 1) * DW1],
                                  in_=w1[k * 128:(k + 1) * 128, :])
                nc.sync.dma_start(out=w2_t[:, k * DW2:(k + 1) * DW2],
                                  in_=w2[k * 128:(k + 1) * 128, :])
            cilo_t = res.tile([128, NBPC * cpl * 8], i16d)
            cihi_t = res.tile([128, NBPC * cph * 8], i16d)
            rl_t = res.tile([128, NBPC * cpe], f16d)
            nc.sync.dma_start(out=cilo_t[:], in_=cilo[:, :])
            nc.sync.dma_start(out=cihi_t[:], in_=cihi[:, :])
            nc.sync.dma_start(out=rl_t[:], in_=rl[:, :])
            ident = res.tile([128, 128], f16d)
            make_identity(nc, ident[:])
            zro5 = res.tile([128, 512], f16d)
            one5 = res.tile([128, 512], f16d)
            nc.gpsimd.memset(zro5[:], 0.0)
            nc.gpsimd.memset(one5[:], 1.0)
            fd_sb = res.tile([128, NBPC * 8], f16d)
            fd2e_sb = res.tile([128, NBPC * cpe], f32d)

            # ---------------- Phase A ----------------
            with nc.named_scope("phaseA"), \
                 tc.tile_pool(name="pa", bufs=3) as pa, \
                 tc.tile_pool(name="ppa", bufs=2, space="PSUM") as ppa:
                for nt in range(NBPC):
                    rows = slice(nt * 128, (nt + 1) * 128)
                    psA = ppa.tile([128, 512], f32d, tag="psA")
                    psB = ppa.tile([128, 16], f32d, tag="psB")
                    xk4 = pa.tile([128, KT * 128], f16d, tag="xk4")
                    for k in range(KT):
                        nc.sync.dma_start(out=xk4[:, k * 128:(k + 1) * 128],
                                          in_=xt[nt * KT + k, :, :])
                    for k in range(KT):
                        nc.tensor.matmul(out=psA[:],
                                         lhsT=xk4[:, k * 128:(k + 1) * 128],
                                         rhs=w1_t[:, k * DW1:k * DW1 + 512],
                                         start=(k == 0), stop=(k == KT - 1))
                        nc.tensor.matmul(out=psB[:],
                                         lhsT=xk4[:, k * 128:(k + 1) * 128],
                                         rhs=w1_t[:, k * DW1 + 512:(k + 1) * DW1],
                                         start=(k == 0), stop=(k == KT - 1))
                    whf = pa.tile([128, DW1], f16d, tag="whf")
                    nc.vector.tensor_copy(out=whf[:, :512], in_=psA[:])
                    nc.vector.tensor_copy(out=whf[:, 512:520], in_=psB[:, 8:16])
                    nc.vector.tensor_tensor(out=whf[:, 520:528], in0=psB[:, 8:16],
                                            in1=whf[:, 512:520], op=ALU.subtract)
                    nc.vector.tensor_copy(out=fd_sb[:, nt * 8:(nt + 1) * 8],
                                          in_=psB[:, 0:8])
                    nc.sync.dma_start(out=tab1s[rows, 0:DW1], in_=whf[:])

            with nc.named_scope("ag1"):
                nc.gpsimd.collective_compute(
                    "AllGather", ALU.bypass, replica_groups=[list(range(NC))],
                    ins=[tab1s[0:HSPL, :]],
                    outs=[tab1a[:].rearrange("(c r) d -> c r d", c=NC)])
                nc.gpsimd.collective_compute(
                    "AllGather", ALU.bypass, replica_groups=[list(range(NC))],
                    ins=[tab1s[HSPL:SHARD, :]],
                    outs=[tab1b[:].rearrange("(c r) d -> c r d", c=NC)])

            # ---------------- Phase B ----------------
            with nc.named_scope("phaseB"), \
                 tc.tile_pool(name="pb", bufs=2) as pb, \
                 tc.tile_pool(name="ppb", bufs=2, space="PSUM") as ppb, \
                 tc.tile_pool(name="ppf", bufs=1, space="PSUM") as ppf, \
                 tc.tile_pool(name="ppt", bufs=1, space="PSUM") as ppt:
                def issue_b(bl):
                    G = pb.tile([128, cpe * TW1], f16d, tag="G")
                    OH = pb.tile([128, cpe * 128], f16d, tag="OH")
                    OHT = pb.tile([128, cpe * 128], f16d, tag="OHT")
                    nc.sync.dma_start(out=OHT[:], in_=ohtT[bl, :, :])
                    nc.sync.dma_start(out=OH[:], in_=ohhT[bl, :, :])
                    gather_pieces(G[:, :cpl * TW1], tab1a[:, :],
                                  cilo_t, bl * cpl * 8, cpl, TW1)
                    gather_pieces(G[:, cpl * TW1:], tab1b[:, :],
                                  cihi_t, bl * cph * 8, cph, TW1)
                    return G, OH, OHT

                pend = issue_b(0)
                for bl in range(NBPC):
                    rows = slice(bl * 128, (bl + 1) * 128)
                    pnum = ppb.tile([128, 512], f32d, tag="pnum")
                    pden = ppb.tile([128, 8], f32d, tag="pden")
                    G, OH, OHT = pend
                    pend = issue_b(bl + 1) if bl + 1 < NBPC else None
                    pfd = ppf.tile([128, cpe * 8], f32d, tag="pfd")
                    for c in range(cpe):
                        nc.tensor.matmul(out=pfd[:, c * 8:(c + 1) * 8],
                                         lhsT=OHT[:, c * 128:(c + 1) * 128],
                                         rhs=fd_sb[:, bl * 8:(bl + 1) * 8],
                                         start=True, stop=True)
                    s1 = pb.tile([128, cpe * 8], f32d, tag="s1")
                    e1 = pb.tile([128, cpe * 8], f32d, tag="e1")
                    p16 = pb.tile([128, cpe * 8], f16d, tag="p16")
                    Gr = G[:].rearrange("p (c d) -> p c d", d=TW1)
                    nc.vector.tensor_tensor(
                        out=s1[:].rearrange("p (c f) -> p c f", c=cpe),
                        in0=Gr[:, :, 512:520],
                        in1=Gr[:, :, 520:528], op=ALU.add)
                    nc.vector.tensor_tensor(out=e1[:], in0=s1[:], in1=pfd[:],
                                            op=ALU.add)
                    nc.vector.tensor_scalar_mul(s1[:], e1[:], ALPHA)
                    nc.vector.tensor_tensor(out=e1[:], in0=e1[:],
                                            in1=s1[:], op=ALU.max)
                    nc.scalar.activation(out=p16[:], in_=e1[:], func=AF.Exp)
                    R = pb.tile([128, cpe * 512], f16d, tag="R")
                    spl = (cpe * 2) // 3
                    nc.vector.tensor_tensor(
                        out=R[:, :spl * 512]
                            .rearrange("p (c e f) -> p c e f", c=spl, e=8),
                        in0=Gr[:, :spl, 0:512]
                            .rearrange("p c (e f) -> p c e f", e=8),
                        in1=p16[:, :spl * 8]
                            .rearrange("p (c h) -> p c h", c=spl)
                            .to_broadcast([128, spl, 8, 64]),
                        op=ALU.mult)
                    nc.gpsimd.tensor_tensor(
                        out=R[:, spl * 512:]
                            .rearrange("p (c e f) -> p c e f", c=cpe - spl, e=8),
                        in0=Gr[:, spl:, 0:512]
                            .rearrange("p c (e f) -> p c e f", e=8),
                        in1=p16[:, spl * 8:]
                            .rearrange("p (c h) -> p c h", c=cpe - spl)
                            .to_broadcast([128, cpe - spl, 8, 64]),
                        op=ALU.mult)
                    # matmuls consume GpSimd-built chunks first? no: vector part
                    # first (chunks 0..spl-1) finishes earlier, matmuls start
                    for i in range(cpe):
                        nc.tensor.matmul(out=pnum[:],
                                         lhsT=OH[:, i * 128:(i + 1) * 128],
                                         rhs=R[:, i * 512:(i + 1) * 512],
                                         start=(i == 0), stop=(i == cpe - 1))
                        nc.tensor.matmul(out=pden[:],
                                         lhsT=OH[:, i * 128:(i + 1) * 128],
                                         rhs=p16[:, i * 8:(i + 1) * 8],
                                         start=(i == 0), stop=(i == cpe - 1))
                    # epilogue: h = elu(num/den), transpose, layer-2 tables
                    dcl = pb.tile([128, 8], f32d, tag="dcl")
                    nc.vector.tensor_scalar_max(dcl[:], pden[:], 1e-30)
                    nc.vector.reciprocal(out=dcl[:], in_=dcl[:])
                    ha = pb.tile([128, 512], f16d, tag="ha")
                    nc.vector.tensor_tensor(
                        out=ha[:].rearrange("p (e f) -> p e f", e=8),
                        in0=pnum[:].rearrange("p (e f) -> p e f", e=8),
                        in1=dcl[:].to_broadcast([128, 8, 64]),
                        op=ALU.mult)
                    hm = pb.tile([128, 512], f16d, tag="hm")
                    nc.vector.tensor_tensor(out=hm[:], in0=ha[:], in1=zro5[:],
                                            op=ALU.min)
                    nc.scalar.activation(out=hm[:], in_=hm[:], func=AF.Exp)
                    nc.vector.tensor_tensor(out=hm[:], in0=hm[:], in1=one5[:],
                                            op=ALU.subtract)
                    h16 = pb.tile([128, 512], f16d, tag="h16")
                    nc.vector.tensor_tensor(out=h16[:], in0=hm[:], in1=ha[:],
                                            op=ALU.max)
                    ps2 = ppt.tile([128, DW2], f32d, tag="ps2")
                    for k in range(KT):
                        pt = ppt.tile([128, 128], f16d, tag="pt")
                        nc.tensor.transpose(out=pt[:],
                                            in_=h16[:, k * 128:(k + 1) * 128],
                                            identity=ident[:])
                        ht = pb.tile([128, 128], f16d, tag="ht")
                        nc.vector.tensor_copy(out=ht[:], in_=pt[:])
                        nc.tensor.matmul(out=ps2[:], lhsT=ht[:],
                                         rhs=w2_t[:, k * DW2:(k + 1) * DW2],
                                         start=(k == 0), stop=(k == KT - 1))
                    t2 = pb.tile([128, DW2], f16d, tag="t2")
                    nc.vector.tensor_copy(out=t2[:, 0:64], in_=ps2[:, 0:64])
                    nc.vector.tensor_copy(out=t2[:, 64:65], in_=ps2[:, 65:66])
                    nc.vector.tensor_tensor(out=t2[:, 65:66], in0=ps2[:, 65:66],
                                            in1=t2[:, 64:65], op=ALU.subtract)
                    fd2 = pb.tile([128, 1], f16d, tag="fd2")
                    nc.vector.tensor_copy(out=fd2[:], in_=ps2[:, 64:65])
                    pfd2 = ppf.tile([128, cpe], f32d, tag="pfd2")
                    for c in range(cpe):
                        nc.tensor.matmul(out=pfd2[:, c:c + 1],
                                         lhsT=OHT[:, c * 128:(c + 1) * 128],
                                         rhs=fd2[:],
                                         start=True, stop=True)
                    nc.vector.tensor_copy(
                        out=fd2e_sb[:, bl * cpe:(bl + 1) * cpe], in_=pfd2[:])
                    nc.sync.dma_start(out=tab2s[rows, 0:DW2], in_=t2[:])

            with nc.named_scope("ag2"):
                nc.gpsimd.collective_compute(
                    "AllGather", ALU.bypass, replica_groups=[list(range(NC))],
                    ins=[tab2s[0:HSPL, :]],
                    outs=[tab2a[:].rearrange("(c r) d -> c r d", c=NC)])
                nc.gpsimd.collective_compute(
                    "AllGather", ALU.bypass, replica_groups=[list(range(NC))],
                    ins=[tab2s[HSPL:SHARD, :]],
                    outs=[tab2b[:].rearrange("(c r) d -> c r d", c=NC)])

            # ---------------- Phase C ----------------
            with nc.named_scope("phaseC"), \
                 tc.tile_pool(name="pc", bufs=3) as pc, \
                 tc.tile_pool(name="ppc", bufs=2, space="PSUM") as ppc:
                def issue_c(bl):
                    G2 = pc.tile([128, cpe * TW2], f16d, tag="G2")
                    OH2 = pc.tile([128, cpe * 128], f16d, tag="OH2")
                    nc.sync.dma_start(out=OH2[:], in_=ohhT[bl, :, :])
                    gather_pieces(G2[:, :cpl * TW2], tab2a[:, :],
                                  cilo_t, bl * cpl * 8, cpl, TW2)
                    gather_pieces(G2[:, cpl * TW2:], tab2b[:, :],
                                  cihi_t, bl * cph * 8, cph, TW2)
                    return G2, OH2

                pend2 = issue_c(0)
                for bl in range(NBPC):
                    rows = slice(bl * 128, (bl + 1) * 128)
                    ps3 = ppc.tile([128, 65], f32d, tag="ps3")
                    G2, OH2 = pend2
                    pend2 = issue_c(bl + 1) if bl + 1 < NBPC else None
                    e2 = pc.tile([128, cpe], f32d, tag="e2")
                    p2 = pc.tile([128, cpe], f16d, tag="p2")
                    G2r = G2[:].rearrange("p (c d) -> p c d", d=TW2)
                    nc.vector.tensor_tensor(
                        out=e2[:].rearrange("p (c o) -> p c o", o=1),
                        in0=G2r[:, :, 64:65],
                        in1=G2r[:, :, 65:66], op=ALU.add)
                    nc.vector.tensor_tensor(
                        out=e2[:], in0=e2[:],
                        in1=fd2e_sb[:, bl * cpe:(bl + 1) * cpe], op=ALU.add)
                    t2c = pc.tile([128, cpe], f32d, tag="t2c")
                    nc.vector.tensor_scalar_mul(t2c[:], e2[:], ALPHA)
                    nc.vector.tensor_tensor(out=e2[:], in0=e2[:],
                                            in1=t2c[:], op=ALU.max)
                    nc.scalar.activation(out=p2[:], in_=e2[:], func=AF.Exp)
                    R2 = pc.tile([128, cpe * 65], f16d, tag="R2")
                    R2r = R2[:].rearrange("p (c d) -> p c d", d=65)
                    nc.vector.tensor_tensor(
                        out=R2r[:, :, 0:64],
                        in0=G2r[:, :, 0:64],
                        in1=p2[:].to_broadcast([128, cpe, 64]),
                        op=ALU.mult)
                    nc.vector.tensor_copy(
                        out=R2r[:, :, 64:65],
                        in_=p2[:].rearrange("p (c o) -> p c o", o=1))
                    for i in range(cpe):
                        nc.tensor.matmul(out=ps3[:],
                                         lhsT=OH2[:, i * 128:(i + 1) * 128],
                                         rhs=R2[:, i * 65:(i + 1) * 65],
                                         start=(i == 0), stop=(i == cpe - 1))
                    d2c = pc.tile([128, 1], f32d, tag="d2c")
                    nc.vector.tensor_scalar_max(d2c[:], ps3[:, 64:65], 1e-30)
                    nc.vector.reciprocal(out=d2c[:], in_=d2c[:])
                    o = pc.tile([128, 64], f32d, tag="o")
                    nc.vector.tensor_tensor(
                        out=o[:].rearrange("p (c f) -> p c f", c=1),
                        in0=ps3[:, 0:64].rearrange("p (c f) -> p c f", c=1),
                        in1=d2c[:].to_broadcast([128, 1, 64]),
                        op=ALU.mult)
                    nc.sync.dma_start(out=out[rows, :], in_=o[:])

    nc.compile()
    _BUILD_CACHE[key] = nc
    return nc


def kernel(**inputs):
    global LAST_EXEC_NS, LAST_RESULTS
    x = inputs["x"].astype(np.float32)
    row = inputs["row"].astype(np.int64)
    col = inputs["col"].astype(np.int64)
    W, a = inputs["W"].astype(np.float32), inputs["a"].astype(np.float32)
    W_out = inputs["W_out"].astype(np.float32)
    a_out = inputs["a_out"].astype(np.float32)

    cilo, cihi, rl, oht, ohh, cpl, cph = _preprocess(row, col)
    cpe = cpl + cph

    W_cat = np.concatenate([W[h] for h in range(NHEADS)], axis=1)
    WA_dst = np.stack([W[h] @ a[h, :NHID] for h in range(NHEADS)], 1)
    WA_src = np.stack([W[h] @ a[h, NHID:] for h in range(NHEADS)], 1)
    w1_np = np.concatenate([W_cat, WA_dst, WA_src], 1).astype(np.float16)
    w2_np = np.concatenate([W_out, (W_out @ a_out[:NCLASS])[:, None],
                            (W_out @ a_out[NCLASS:])[:, None]], 1).astype(np.float16)

    x_pad = np.zeros((NPAD, NFEAT), np.float16)
    x_pad[:N] = x

    nc = _build(cpl, cph)

    in_maps = []
    for c in range(NC):
        xs = x_pad[c * SHARD:(c + 1) * SHARD]            # [6272, 512]
        xt = (xs.reshape(NBPC, 128, KT, 128)             # [nt, n, k, f]
                .transpose(0, 2, 3, 1)                   # [nt, k, f, n]
                .reshape(NBPC * KT, 128, 128)).copy()
        in_maps.append({"xt": xt, "w1": w1_np, "w2": w2_np,
                        "cilo": cilo[c], "cihi": cihi[c],
                        "rl": rl[c], "ohtT": oht[c], "ohhT": ohh[c]})

    trace = bool(int(os.environ.get("GAT_TRACE", "0")))
    res = run_bass_kernel_spmd(nc, in_maps, list(range(NC)), trace=trace,
                               trace_cores=list(range(NC)) if trace else None)
    LAST_EXEC_NS = res.exec_time_ns
    LAST_RESULTS = res
    outs = [res.results[c]["out"] for c in range(NC)]
    return np.concatenate(outs, 0)[:N].astype(np.float32)
